# revision 1
# baseline (speedup 1.0000x reference)
"""AFT-Full on 8 TRN2 cores — raw Bacc build (no TileContext).

Same algorithm/layout as kernel.py v13 (see its docstring), but with
hand-managed semaphores so the NEFF tail is a single short barrier
instead of Tile's ~6us lazy-semaphore cleanup storm.

Engine streams (per core, 4 segments of widths 128/384/256/256):
  SYNC   : 3 input DMAs, 4 output DMAs, final completion wait
  SCALAR : weight DMA, per-seg exp + 2 PSUM->SBUF copies
  TENSOR : per-seg 10 matmuls (k/q/v accum pairs, den/num, 2 finals)
  VECTOR : per-seg ekv, reciprocal, r, o1

Semaphores: SX0/SX1/SX2 (one per input DMA), SW (w DMA), SO0 (first
out DMA), SO (remaining out DMAs), SP/SA/SV (matmul/ACT/DVE ops, +1
each).  CRITICAL RULE: a DMA's then_inc(sem,16) is sixteen +1s from
sixteen queue engines that do NOT finish in lockstep, so a shared
counter can satisfy wait_ge(16) with a MIX of increments from two
DMAs while neither is complete.  Every DMA-completion wait therefore
references either a single-DMA semaphore or an exact all-DMAs total.
WAR hazards on the static PSUM banks and SBUF tensors are covered by
the thresholds derived in comments below.
"""

import os
import sys

sys.path.insert(0, "/opt/trn_rl_repo")

import numpy as np

from concourse import bacc, mybir
from concourse.bass_utils import run_bass_kernel_spmd

BS, C, HH, WW = 4, 128, 64, 64
T = HH * WW
IC = C // 2
NCORES = 8
NCOL = BS * T // NCORES   # 2048
F = 512
HF = F // 2
LEAD = 128
REST = F - LEAD
WC = 768

_f32 = mybir.dt.float32
_bf16 = mybir.dt.bfloat16

_cached = {}


def _install_ntff_hook():
    import types

    if "antenv.axon_hooks" in sys.modules:
        return
    mod = types.ModuleType("antenv.axon_hooks")
    state = {"hook": None}
    mod.set_axon_ntff_profile_hook = lambda h: state.update(hook=h)
    mod.get_axon_ntff_profile_hook = lambda: state["hook"]
    sys.modules["antenv.axon_hooks"] = mod
    try:
        sys.path.insert(0, "/root/.axon_site")
        from trn_agent_boot.trn_boot import _ntff_profile_via_ctypes

        hook = _ntff_profile_via_ctypes("/opt/axon/libaxon_pjrt.so")
        if hook is not None:
            mod.set_axon_ntff_profile_hook(hook)
    except Exception as e:
        print(f"ntff hook install failed: {e}", file=sys.stderr)


# segment table: (width, x-col base A, out block pair base)
# seg i covers columns [A: blkA*512 + c0 : +wd][B: blkB*512 + c0 : +wd]
SEGS = [
    (LEAD, 0),    # cols A 0:128    B 512:640
    (REST, LEAD), # cols A 128:512  B 640:1024
    (HF, 0),      # cols A 1024:1280 B 1536:1792
    (HF, HF),     # cols A 1280:1536 B 1792:2048
]


def _build():
    nc = bacc.Bacc("TRN2", target_bir_lowering=False, debug=False)
    x_ext = nc.dram_tensor("x", [C, NCOL], _bf16, kind="ExternalInput")
    w_ext = nc.dram_tensor("w", [C, WC], _bf16, kind="ExternalInput")
    out_ext = nc.dram_tensor("out", [C, NCOL], _f32, kind="ExternalOutput")

    EXP = mybir.ActivationFunctionType.Exp

    # static SBUF tensors
    xlead = nc.alloc_sbuf_tensor("xlead", [C, 2, LEAD], _bf16)
    xrest = nc.alloc_sbuf_tensor("xrest", [C, 2, REST], _bf16)
    x23 = nc.alloc_sbuf_tensor("x23", [C, 2 * F], _bf16)
    w = nc.alloc_sbuf_tensor("w_sb", [C, WC], _bf16)
    ek = nc.alloc_sbuf_tensor("ek", [C, F], _bf16)
    ekv = nc.alloc_sbuf_tensor("ekv", [C, F], _bf16)
    rden = nc.alloc_sbuf_tensor("rden", [C, F], _f32)
    rr = nc.alloc_sbuf_tensor("rr", [C, F], _f32)
    o1 = nc.alloc_sbuf_tensor("o1", [C, F], _bf16)
    ot = [
        nc.alloc_sbuf_tensor("ot0", [C, 2, F], _f32),
        nc.alloc_sbuf_tensor("ot1", [C, 2, F], _f32),
        nc.alloc_sbuf_tensor("ot2", [C, 2, F], _f32),
    ]

    # static PSUM banks
    pk = nc.alloc_psum_tensor("pk", [C, F], _f32)
    pq0 = nc.alloc_psum_tensor("pq0", [C, F], _f32)
    pq1 = nc.alloc_psum_tensor("pq1", [C, F], _f32)
    pqs = [pq0, pq1]
    pv = nc.alloc_psum_tensor("pv", [C, F], _f32)
    pden = nc.alloc_psum_tensor("pden", [C, F], _f32)
    pnum = nc.alloc_psum_tensor("pnum", [C, F], _f32)
    poa = nc.alloc_psum_tensor("poa", [C, F], _f32)
    pob = nc.alloc_psum_tensor("pob", [C, F], _f32)

    SX0 = nc.alloc_semaphore("SX0")
    SX1 = nc.alloc_semaphore("SX1")
    SX2 = nc.alloc_semaphore("SX2")
    SW = nc.alloc_semaphore("SW")
    SO0 = nc.alloc_semaphore("SO0")
    SP = nc.alloc_semaphore("SP")
    SA = nc.alloc_semaphore("SA")
    SV = nc.alloc_semaphore("SV")
    SO = nc.alloc_semaphore("SO")

    # weight slices: [Z WkT Z | Z WqT Z WvT Z | eB-blkdiag | WmT x2]
    wk = (w[:, 64:192], w[:, 0:128])
    wq = (w[:, 256:384], w[:, 192:320])
    wv = (w[:, 384:512], w[:, 320:448])
    w_eB = w[:, 512:640]
    w_m = w[:, 640:768]

    xv = x_ext[:].rearrange("p (b c) -> p b c", b=4)
    ov = out_ext[:].rearrange("p (b c) -> p b c", b=4)

    def xab(i):
        wd, c0 = SEGS[i]
        if i == 0:
            return xlead[:, 0, :], xlead[:, 1, :]
        if i == 1:
            return xrest[:, 0, :], xrest[:, 1, :]
        return x23[:, c0:c0 + wd], x23[:, F + c0:F + c0 + wd]

    # --- software-pipelined PE schedule -------------------------------
    # PE order: kqv_0, den_0, num_0, [kqv_1, fin_0, den_1, num_1], ...,
    # fin_3.  fin_i is deferred into segment i+1's slot so the PE has
    # k/q/v work while segment i's DVE chain (recip, r, o1) completes.
    # SP positions (cumulative matmul count) for each milestone:
    NS = len(SEGS)
    k_done, v_done, den_pos, num_pos = {}, {}, {}, {}
    fa_pos, fb_pos = {}, {}
    pos = 0
    for i in range(NS):
        pos += 2
        k_done[i] = pos
        pos += 2  # q
        pos += 2
        v_done[i] = pos
        if i > 0:
            fa_pos[i - 1] = pos + 1
            fb_pos[i - 1] = pos + 2
            pos += 2
        den_pos[i] = pos + 1
        num_pos[i] = pos + 2
        pos += 2
    fa_pos[NS - 1] = pos + 1
    fb_pos[NS - 1] = pos + 2
    # SA stream order: exp0, [exp_{i}, ota_{i-1}, otb_{i-1}] ..., ota3, otb3
    exp_done, ota_done, otb_done = {}, {}, {}
    sa = 1
    exp_done[0] = sa
    for i in range(1, NS):
        sa += 1; exp_done[i] = sa
        sa += 1; ota_done[i - 1] = sa
        sa += 1; otb_done[i - 1] = sa
    sa += 1; ota_done[NS - 1] = sa
    sa += 1; otb_done[NS - 1] = sa

    with nc.Block() as block:

        @block.sync
        def _(sync):
            sync.dma_start(xlead[:], xv[:, 0:2, 0:LEAD]).then_inc(SX0, 16)
            sync.dma_start(xrest[:], xv[:, 0:2, LEAD:F]).then_inc(SX1, 16)
            sync.dma_start(x23[:], x_ext[:, 2 * F:4 * F]).then_inc(SX2, 16)
            for i, (wd, c0) in enumerate(SEGS[:-1]):
                blk = 0 if i < 2 else 2
                sync.wait_ge(SA, otb_done[i])
                sync.dma_start(
                    ov[:, blk:blk + 2, c0:c0 + wd], ot[i % 3][:, 0:2, 0:wd]
                ).then_inc(SO0 if i == 0 else SO, 16)
            # tail segment: b-half on this ring as soon as the DVE copy
            # lands (a-half goes out on the scalar ring)
            i, (wd, c0) = NS - 1, SEGS[NS - 1]
            sync.wait_ge(SV, 4 * NS + 1)  # otb3 copy (on DVE)
            sync.dma_start(
                ov[:, 3, c0:c0 + wd], ot[i % 3][:, 1, 0:wd]
            ).then_inc(SO, 16)
            sync.wait_ge(SO0, 16)
            sync.wait_ge(SO, 64)

        @block.gpsimd
        def _(gpsimd):
            # restore semaphores for potential NEFF re-execution
            gpsimd.wait_ge(SO0, 16)
            gpsimd.wait_ge(SO, 64)
            gpsimd.sem_clear(range(SX0.num, SO.num + 1))

        @block.scalar
        def _(scalar):
            scalar.dma_start(w[:], w_ext[:]).then_inc(SW, 16)

            def exp_op(i):
                wd = SEGS[i][0]
                scalar.wait_ge(SP, k_done[i])
                scalar.activation(ek[:, 0:wd], pk[:, 0:wd], EXP).then_inc(SA)

            def copies(i):
                wd = SEGS[i][0]
                scalar.wait_ge(SP, fa_pos[i])
                scalar.copy(ot[i % 3][:, 0, 0:wd], poa[:, 0:wd]).then_inc(SA)
                scalar.wait_ge(SP, fb_pos[i])
                scalar.copy(ot[i % 3][:, 1, 0:wd], pob[:, 0:wd]).then_inc(SA)

            exp_op(0)
            for i in range(1, len(SEGS)):
                exp_op(i)
                copies(i - 1)
            # tail segment: only the a-half copy here (b-half on DVE),
            # then its out-DMA on this ring.
            i, wd = NS - 1, SEGS[NS - 1][0]
            scalar.wait_ge(SP, fa_pos[i])
            scalar.wait_ge(SO0, 16)  # WAR: ot[0] read by seg-0 DMA
            scalar.copy(ot[i % 3][:, 0, 0:wd], poa[:, 0:wd]).then_inc(SA)
            scalar.dma_start(
                ov[:, 2, SEGS[i][1]:SEGS[i][1] + wd], ot[i % 3][:, 0, 0:wd]
            ).then_inc(SO, 16)

        @block.tensor
        def _(tensor):
            def kqv(i):
                wd, c0 = SEGS[i]
                xa, xb = xab(i)
                if i == 0:
                    tensor.wait_ge(SW, 16)
                    tensor.wait_ge(SX0, 16)
                else:
                    if i == 1:
                        tensor.wait_ge(SX1, 16)
                    elif i == 2:
                        tensor.wait_ge(SX2, 16)
                    tensor.wait_ge(SA, exp_done[i - 1])  # WAR pk vs exp
                tensor.matmul(pk[:, 0:wd], wk[0], xa, start=True, stop=False
                              ).then_inc(SP)
                tensor.matmul(pk[:, 0:wd], wk[1], xb, start=False, stop=True
                              ).then_inc(SP)
                if i >= 2:
                    tensor.wait_ge(SV, 4 * (i - 1))  # WAR pq[i%2] vs o1(i-2)
                pq = pqs[i % 2]
                tensor.matmul(pq[:, 0:wd], wq[0], xa, start=True, stop=False
                              ).then_inc(SP)
                tensor.matmul(pq[:, 0:wd], wq[1], xb, start=False, stop=True
                              ).then_inc(SP)
                if i >= 1:
                    tensor.wait_ge(SV, 4 * (i - 1) + 1)  # WAR pv vs ekv(i-1)
                tensor.matmul(pv[:, 0:wd], wv[0], xa, start=True, stop=False
                              ).then_inc(SP)
                tensor.matmul(pv[:, 0:wd], wv[1], xb, start=False, stop=True
                              ).then_inc(SP)

            def dennum(i):
                wd, c0 = SEGS[i]
                tensor.wait_ge(SA, exp_done[i])  # ek ready (covers WAR)
                if i >= 1:
                    tensor.wait_ge(SV, 4 * (i - 1) + 2)  # WAR pden
                tensor.matmul(pden[:, 0:wd], w_eB, ek[:, 0:wd]).then_inc(SP)
                tensor.wait_ge(SV, 4 * i + 1)  # ekv ready (covers WAR pnum)
                tensor.matmul(pnum[:, 0:wd], w_eB, ekv[:, 0:wd]).then_inc(SP)

            def fins(i):
                wd, c0 = SEGS[i]
                tensor.wait_ge(SV, 4 * i + 4)  # o1 ready
                if i >= 1:
                    tensor.wait_ge(SA, ota_done[i - 1])  # WAR poa vs ota
                tensor.matmul(poa[:, 0:wd], w_m[0:64, :], o1[0:64, 0:wd]
                              ).then_inc(SP)
                if i >= 1:
                    tensor.wait_ge(SA, otb_done[i - 1])  # WAR pob vs otb
                tensor.matmul(pob[:, 0:wd], w_m[64:128, :], o1[64:128, 0:wd]
                              ).then_inc(SP)

            for i in range(len(SEGS)):
                kqv(i)
                if i > 0:
                    fins(i - 1)
                dennum(i)
            fins(len(SEGS) - 1)

        @block.vector
        def _(vector):
            for i, (wd, c0) in enumerate(SEGS):
                vector.wait_ge(SA, exp_done[i])
                vector.wait_ge(SP, v_done[i])
                vector.tensor_mul(ekv[:, 0:wd], ek[:, 0:wd], pv[:, 0:wd]
                                  ).then_inc(SV)
                vector.wait_ge(SP, den_pos[i])
                vector.reciprocal_approx_fast(rden[:, 0:wd], pden[:, 0:wd]
                                              ).then_inc(SV)
                vector.wait_ge(SP, num_pos[i])
                vector.tensor_mul(rr[:, 0:wd], rden[:, 0:wd], pnum[:, 0:wd]
                                  ).then_inc(SV)
                vector.tensor_mul(o1[:, 0:wd], rr[:, 0:wd],
                                  pqs[i % 2][:, 0:wd]).then_inc(SV)
                if i == len(SEGS) - 1:
                    vector.wait_ge(SP, fb_pos[i])
                    vector.wait_ge(SO0, 16)  # WAR: ot[0] read by seg-0 DMA
                    vector.tensor_copy(ot[i % 3][:, 1, 0:wd], pob[:, 0:wd]
                                       ).then_inc(SV)

    nc.compile()
    return nc


def _pack_weights(Wq, Wk, Wv, B, Wm):
    import ml_dtypes

    eB = np.exp(B)
    w = np.zeros((C, WC), np.float32)
    w[:, 64:128] = Wk.T
    w[:, 256:320] = Wq.T
    w[:, 384:448] = Wv.T
    w[0:IC, 512:576] = eB.T
    w[IC:C, 576:640] = eB.T
    w[0:IC, 640:768] = Wm.T
    w[IC:C, 640:768] = Wm.T
    return np.ascontiguousarray(w.astype(ml_dtypes.bfloat16))


def kernel(x, Wq, Wk, Wv, B, Wm):
    import ml_dtypes

    x = np.ascontiguousarray(np.asarray(x, dtype=np.float32))
    Wq = np.asarray(Wq, dtype=np.float32)
    Wk = np.asarray(Wk, dtype=np.float32)
    Wv = np.asarray(Wv, dtype=np.float32)
    B = np.asarray(B, dtype=np.float32)
    Wm = np.asarray(Wm, dtype=np.float32)

    xf = x.reshape(BS, C, T)
    per_batch = NCORES // BS
    shards = []
    for core in range(NCORES):
        b, j = divmod(core, per_batch)
        shards.append(np.ascontiguousarray(
            xf[b, :, j * NCOL:(j + 1) * NCOL].astype(ml_dtypes.bfloat16)))

    w = _pack_weights(Wq, Wk, Wv, B, Wm)

    if "nc" not in _cached:
        _cached["nc"] = _build()
    nc = _cached["nc"]

    in_maps = [{"x": shards[i], "w": w} for i in range(NCORES)]
    trace = bool(int(os.environ.get("AFT_TRACE", "0")))
    if trace:
        _install_ntff_hook()
    try:
        res = run_bass_kernel_spmd(
            nc, in_maps, core_ids=list(range(NCORES)), trace=trace
        )
    except Exception as e:  # rare transient device wedge: retry once
        print(f"run_bass_kernel_spmd failed ({e}); retrying", file=sys.stderr)
        import time

        time.sleep(3.0)
        res = run_bass_kernel_spmd(
            nc, in_maps, core_ids=list(range(NCORES)), trace=trace
        )
    kernel.last_exec_time_ns = res.exec_time_ns
    kernel.last_results = res

    out = np.empty((BS, C, T), np.float32)
    for core in range(NCORES):
        b, j = divmod(core, per_batch)
        out[b, :, j * NCOL:(j + 1) * NCOL] = res.results[core]["out"]
    return out.reshape(BS, C, HH, WW)


kernel.last_exec_time_ns = None
kernel.last_results = None



# revision 3
# speedup vs baseline: 1.0426x; 1.0426x over previous
"""AFT-Full on 8 TRN2 cores — raw Bacc build (no TileContext).

v14 over the v13 baseline (29.5us -> target ~20us):
  * input DMA posts SPLICED to the top of each engine's instruction
    stream, ahead of the framework's all-engine barrier, so the DGE
    transfers fly while the preamble (register loads, memsets, act
    table) is still running.  Saves ~2.5-3us of head latency.
  * outputs written as bf16 (halves output HBM bytes; rel-err budget
    2e-2 >> the ~3e-3 bf16 rounding adds).
  * weight DMA split in two (kqv block first) and repacked to 704
    cols so the first matmuls' weights land sooner.
  * segment widths reordered 128/384/384/128 so the serial drain-down
    tail (vector chain -> fins -> copies -> out DMA) runs on a small
    segment.

Engine streams (per core, 4 segments):
  SYNC   : xlead+xrest input DMAs (spliced early), 4 output DMAs,
           final completion wait
  SCALAR : w_a/w_b/x23 DMAs (spliced early), per-seg exp + 2
           PSUM->SBUF copies
  TENSOR : per-seg 10 matmuls (k/q/v accum pairs, den/num, 2 finals)
  VECTOR : per-seg ekv, reciprocal, r, o1

Semaphores: SX0/SX1/SX2 (one per input DMA), SWA/SWB (w DMAs), SO0
(first out DMA), SO (remaining out DMAs), SP/SA/SV (matmul/ACT/DVE
ops, +1 each).  CRITICAL RULE: a DMA's then_inc(sem,16) is sixteen
+1s from sixteen queue engines that do NOT finish in lockstep, so a
shared counter can satisfy wait_ge(16) with a MIX of increments from
two DMAs while neither is complete.  Every DMA-completion wait
therefore references either a single-DMA semaphore or an exact
all-DMAs total.  WAR hazards on the static PSUM banks and SBUF
tensors are covered by the thresholds derived in comments below.
"""

import os
import sys

sys.path.insert(0, "/opt/trn_rl_repo")

import numpy as np

from concourse import bacc, mybir
from concourse.bass_utils import run_bass_kernel_spmd

BS, C, HH, WW = 4, 128, 64, 64
T = HH * WW
IC = C // 2
NCORES = 8
NCOL = BS * T // NCORES   # 2048
F = 512
LEAD = 128
REST = F - LEAD
WC = 704          # packed weight cols: [Z K Z Q Z V Z](448) eB(128) Wm(128)
WA = 448          # first weight DMA chunk (kqv)

_f32 = mybir.dt.float32
_bf16 = mybir.dt.bfloat16

_cached = {}


def _install_ntff_hook():
    import types

    if "antenv.axon_hooks" in sys.modules:
        return
    mod = types.ModuleType("antenv.axon_hooks")
    state = {"hook": None}
    mod.set_axon_ntff_profile_hook = lambda h: state.update(hook=h)
    mod.get_axon_ntff_profile_hook = lambda: state["hook"]
    sys.modules["antenv.axon_hooks"] = mod
    try:
        sys.path.insert(0, "/root/.axon_site")
        from trn_agent_boot.trn_boot import _ntff_profile_via_ctypes

        hook = _ntff_profile_via_ctypes("/opt/axon/libaxon_pjrt.so")
        if hook is not None:
            mod.set_axon_ntff_profile_hook(hook)
    except Exception as e:
        print(f"ntff hook install failed: {e}", file=sys.stderr)


# segment table: (width, col base within the 512-wide block)
# seg 0/1 cover block pair (0,1); seg 2/3 cover pair (2,3).
SEGS = [
    (LEAD, 0),    # cols A 0:128     B 512:640
    (REST, LEAD), # cols A 128:512   B 640:1024
    (REST, 0),    # cols A 1024:1408 B 1536:1920
    (LEAD, REST), # cols A 1408:1536 B 1920:2048
]


def _splice_early(nc, early):
    """Move the captured input-DMA instructions to the top of each
    engine's stream, right after that engine's leading barrier Drain,
    so the transfers start during the framework preamble instead of
    after the all-engine barrier."""
    raw = [bi.ins for bi in early]
    raw_ids = {id(r) for r in raw}
    f = nc.main_func
    for b in f.blocks:
        b.instructions[:] = [i for i in b.instructions if id(i) not in raw_ids]
    entry = f.blocks[0]
    ins_pt = {}
    for idx, ins in enumerate(entry.instructions):
        if isinstance(ins, mybir.InstDrain) and ins.engine not in ins_pt:
            ins_pt[ins.engine] = idx + 1
    for r in raw:
        at = ins_pt.get(r.engine, 0)
        entry.instructions.insert(at, r)
        if r.engine not in ins_pt:
            ins_pt[r.engine] = 0
        for e in ins_pt:
            if ins_pt[e] >= at:
                ins_pt[e] += 1


def _build():
    nc = bacc.Bacc("TRN2", target_bir_lowering=False, debug=False)
    x_ext = nc.dram_tensor("x", [C, NCOL], _bf16, kind="ExternalInput")
    w_ext = nc.dram_tensor("w", [C, WC], _bf16, kind="ExternalInput")
    out_ext = nc.dram_tensor("out", [C, NCOL], _bf16, kind="ExternalOutput")

    EXP = mybir.ActivationFunctionType.Exp

    # static SBUF tensors
    xlead = nc.alloc_sbuf_tensor("xlead", [C, 2, LEAD], _bf16)
    xrest = nc.alloc_sbuf_tensor("xrest", [C, 2, REST], _bf16)
    x23 = nc.alloc_sbuf_tensor("x23", [C, 2 * F], _bf16)
    w = nc.alloc_sbuf_tensor("w_sb", [C, WC], _bf16)
    ek = nc.alloc_sbuf_tensor("ek", [C, F], _bf16)
    ekv = nc.alloc_sbuf_tensor("ekv", [C, F], _bf16)
    rden = nc.alloc_sbuf_tensor("rden", [C, F], _f32)
    rr = nc.alloc_sbuf_tensor("rr", [C, F], _f32)
    o1 = nc.alloc_sbuf_tensor("o1", [C, F], _bf16)
    ot = [
        nc.alloc_sbuf_tensor("ot0", [C, 2, F], _bf16),
        nc.alloc_sbuf_tensor("ot1", [C, 2, F], _bf16),
        nc.alloc_sbuf_tensor("ot2", [C, 2, F], _bf16),
    ]

    # static PSUM banks
    pk = nc.alloc_psum_tensor("pk", [C, F], _f32)
    pq0 = nc.alloc_psum_tensor("pq0", [C, F], _f32)
    pq1 = nc.alloc_psum_tensor("pq1", [C, F], _f32)
    pqs = [pq0, pq1]
    pv = nc.alloc_psum_tensor("pv", [C, F], _f32)
    pden = nc.alloc_psum_tensor("pden", [C, F], _f32)
    pnum = nc.alloc_psum_tensor("pnum", [C, F], _f32)
    poa = nc.alloc_psum_tensor("poa", [C, F], _f32)
    pob = nc.alloc_psum_tensor("pob", [C, F], _f32)

    SX0 = nc.alloc_semaphore("SX0")
    SX1 = nc.alloc_semaphore("SX1")
    SX2 = nc.alloc_semaphore("SX2")
    SWA = nc.alloc_semaphore("SWA")
    SWB = nc.alloc_semaphore("SWB")
    SO0 = nc.alloc_semaphore("SO0")
    SP = nc.alloc_semaphore("SP")
    SA = nc.alloc_semaphore("SA")
    SV = nc.alloc_semaphore("SV")
    SO = nc.alloc_semaphore("SO")

    # weight slices in the 704-col pack:
    # [Z(64) WkT(64) Z(64) WqT(64) Z(64) WvT(64) Z(64) | eB-blkdiag(128) | WmT x2 (128)]
    wk = (w[:, 64:192], w[:, 0:128])
    wq = (w[:, 192:320], w[:, 128:256])
    wv = (w[:, 320:448], w[:, 256:384])
    w_eB = w[:, 448:576]
    w_m = w[:, 576:704]

    xv = x_ext[:].rearrange("p (b c) -> p b c", b=4)
    ov = out_ext[:].rearrange("p (b c) -> p b c", b=4)

    def xab(i):
        wd, c0 = SEGS[i]
        if i == 0:
            return xlead[:, 0, :], xlead[:, 1, :]
        if i == 1:
            return xrest[:, 0, :], xrest[:, 1, :]
        return x23[:, c0:c0 + wd], x23[:, F + c0:F + c0 + wd]

    # --- software-pipelined PE schedule -------------------------------
    # PE order: kqv_0, den_0, num_0, [kqv_1, fin_0, den_1, num_1], ...,
    # fin_3.  fin_i is deferred into segment i+1's slot so the PE has
    # k/q/v work while segment i's DVE chain (recip, r, o1) completes.
    # SP positions (cumulative matmul count) for each milestone:
    NS = len(SEGS)
    k_done, v_done, den_pos, num_pos = {}, {}, {}, {}
    fa_pos, fb_pos = {}, {}
    pos = 0
    for i in range(NS):
        pos += 2
        k_done[i] = pos
        pos += 2  # q
        pos += 2
        v_done[i] = pos
        if i > 0:
            fa_pos[i - 1] = pos + 1
            fb_pos[i - 1] = pos + 2
            pos += 2
        den_pos[i] = pos + 1
        num_pos[i] = pos + 2
        pos += 2
    fa_pos[NS - 1] = pos + 1
    fb_pos[NS - 1] = pos + 2
    # SA stream order: exp0, [exp_{i}, ota_{i-1}, otb_{i-1}] ..., ota3, otb3
    exp_done, ota_done, otb_done = {}, {}, {}
    sa = 1
    exp_done[0] = sa
    for i in range(1, NS):
        sa += 1; exp_done[i] = sa
        sa += 1; ota_done[i - 1] = sa
        sa += 1; otb_done[i - 1] = sa
    sa += 1; ota_done[NS - 1] = sa
    sa += 1; otb_done[NS - 1] = sa

    early = []  # input DMA posts to splice ahead of the barrier

    with nc.Block() as block:

        @block.sync
        def _(sync):
            early.append(
                sync.dma_start(xlead[:], xv[:, 0:2, 0:LEAD]).then_inc(SX0, 16))
            early.append(
                sync.dma_start(xrest[:], xv[:, 0:2, LEAD:F]).then_inc(SX1, 16))
            for i, (wd, c0) in enumerate(SEGS[:-1]):
                blk = 0 if i < 2 else 2
                sync.wait_ge(SA, otb_done[i])
                sync.dma_start(
                    ov[:, blk:blk + 2, c0:c0 + wd], ot[i % 3][:, 0:2, 0:wd]
                ).then_inc(SO0 if i == 0 else SO, 16)
            # tail segment: b-half on this ring as soon as the DVE copy
            # lands (a-half goes out on the scalar ring)
            i, (wd, c0) = NS - 1, SEGS[NS - 1]
            sync.wait_ge(SV, 4 * NS + 1)  # otb3 copy (on DVE)
            sync.dma_start(
                ov[:, 3, c0:c0 + wd], ot[i % 3][:, 1, 0:wd]
            ).then_inc(SO, 16)
            sync.wait_ge(SO0, 16)
            sync.wait_ge(SO, 64)

        @block.gpsimd
        def _(gpsimd):
            # restore semaphores for potential NEFF re-execution
            gpsimd.wait_ge(SO0, 16)
            gpsimd.wait_ge(SO, 64)
            gpsimd.sem_clear(range(SX0.num, SO.num + 1))

        @block.scalar
        def _(scalar):
            early.append(
                scalar.dma_start(w[:, 0:WA], w_ext[:, 0:WA]).then_inc(SWA, 16))
            early.append(
                scalar.dma_start(w[:, WA:WC], w_ext[:, WA:WC]).then_inc(SWB, 16))
            early.append(
                scalar.dma_start(x23[:], x_ext[:, 2 * F:4 * F]).then_inc(SX2, 16))

            def exp_op(i):
                wd = SEGS[i][0]
                scalar.wait_ge(SP, k_done[i])
                scalar.activation(ek[:, 0:wd], pk[:, 0:wd], EXP).then_inc(SA)

            def copies(i):
                wd = SEGS[i][0]
                scalar.wait_ge(SP, fa_pos[i])
                scalar.copy(ot[i % 3][:, 0, 0:wd], poa[:, 0:wd]).then_inc(SA)
                scalar.wait_ge(SP, fb_pos[i])
                scalar.copy(ot[i % 3][:, 1, 0:wd], pob[:, 0:wd]).then_inc(SA)

            exp_op(0)
            for i in range(1, len(SEGS)):
                exp_op(i)
                copies(i - 1)
            # tail segment: only the a-half copy here (b-half on DVE),
            # then its out-DMA on this ring.
            i, wd = NS - 1, SEGS[NS - 1][0]
            scalar.wait_ge(SP, fa_pos[i])
            scalar.wait_ge(SO0, 16)  # WAR: ot[0] read by seg-0 DMA
            scalar.copy(ot[i % 3][:, 0, 0:wd], poa[:, 0:wd]).then_inc(SA)
            scalar.dma_start(
                ov[:, 2, SEGS[i][1]:SEGS[i][1] + wd], ot[i % 3][:, 0, 0:wd]
            ).then_inc(SO, 16)

        @block.tensor
        def _(tensor):
            def kqv(i):
                wd, c0 = SEGS[i]
                xa, xb = xab(i)
                if i == 0:
                    tensor.wait_ge(SWA, 16)
                    tensor.wait_ge(SX0, 16)
                else:
                    if i == 1:
                        tensor.wait_ge(SX1, 16)
                    elif i == 2:
                        tensor.wait_ge(SX2, 16)
                    tensor.wait_ge(SA, exp_done[i - 1])  # WAR pk vs exp
                tensor.matmul(pk[:, 0:wd], wk[0], xa, start=True, stop=False
                              ).then_inc(SP)
                tensor.matmul(pk[:, 0:wd], wk[1], xb, start=False, stop=True
                              ).then_inc(SP)
                if i >= 2:
                    tensor.wait_ge(SV, 4 * (i - 1))  # WAR pq[i%2] vs o1(i-2)
                pq = pqs[i % 2]
                tensor.matmul(pq[:, 0:wd], wq[0], xa, start=True, stop=False
                              ).then_inc(SP)
                tensor.matmul(pq[:, 0:wd], wq[1], xb, start=False, stop=True
                              ).then_inc(SP)
                if i >= 1:
                    tensor.wait_ge(SV, 4 * (i - 1) + 1)  # WAR pv vs ekv(i-1)
                tensor.matmul(pv[:, 0:wd], wv[0], xa, start=True, stop=False
                              ).then_inc(SP)
                tensor.matmul(pv[:, 0:wd], wv[1], xb, start=False, stop=True
                              ).then_inc(SP)

            def dennum(i):
                wd, c0 = SEGS[i]
                tensor.wait_ge(SA, exp_done[i])  # ek ready (covers WAR)
                if i == 0:
                    tensor.wait_ge(SWB, 16)      # eB/Wm weights landed
                if i >= 1:
                    tensor.wait_ge(SV, 4 * (i - 1) + 2)  # WAR pden
                tensor.matmul(pden[:, 0:wd], w_eB, ek[:, 0:wd]).then_inc(SP)
                tensor.wait_ge(SV, 4 * i + 1)  # ekv ready (covers WAR pnum)
                tensor.matmul(pnum[:, 0:wd], w_eB, ekv[:, 0:wd]).then_inc(SP)

            def fins(i):
                wd, c0 = SEGS[i]
                tensor.wait_ge(SV, 4 * i + 4)  # o1 ready
                if i >= 1:
                    tensor.wait_ge(SA, ota_done[i - 1])  # WAR poa vs ota
                tensor.matmul(poa[:, 0:wd], w_m[0:64, :], o1[0:64, 0:wd]
                              ).then_inc(SP)
                if i >= 1:
                    tensor.wait_ge(SA, otb_done[i - 1])  # WAR pob vs otb
                tensor.matmul(pob[:, 0:wd], w_m[64:128, :], o1[64:128, 0:wd]
                              ).then_inc(SP)

            for i in range(len(SEGS)):
                kqv(i)
                if i > 0:
                    fins(i - 1)
                dennum(i)
            fins(len(SEGS) - 1)

        @block.vector
        def _(vector):
            for i, (wd, c0) in enumerate(SEGS):
                vector.wait_ge(SA, exp_done[i])
                vector.wait_ge(SP, v_done[i])
                vector.tensor_mul(ekv[:, 0:wd], ek[:, 0:wd], pv[:, 0:wd]
                                  ).then_inc(SV)
                vector.wait_ge(SP, den_pos[i])
                vector.reciprocal_approx_fast(rden[:, 0:wd], pden[:, 0:wd]
                                              ).then_inc(SV)
                vector.wait_ge(SP, num_pos[i])
                vector.tensor_mul(rr[:, 0:wd], rden[:, 0:wd], pnum[:, 0:wd]
                                  ).then_inc(SV)
                vector.tensor_mul(o1[:, 0:wd], rr[:, 0:wd],
                                  pqs[i % 2][:, 0:wd]).then_inc(SV)
                if i == len(SEGS) - 1:
                    vector.wait_ge(SP, fb_pos[i])
                    vector.wait_ge(SO0, 16)  # WAR: ot[0] read by seg-0 DMA
                    vector.tensor_copy(ot[i % 3][:, 1, 0:wd], pob[:, 0:wd]
                                       ).then_inc(SV)

    if bool(int(os.environ.get("AFT_SPLICE", "1"))):
        _splice_early(nc, early)

    nc.compile()
    return nc


def _pack_weights(Wq, Wk, Wv, B, Wm):
    import ml_dtypes

    eB = np.exp(B)
    w = np.zeros((C, WC), np.float32)
    w[:, 64:128] = Wk.T
    w[:, 192:256] = Wq.T
    w[:, 320:384] = Wv.T
    w[0:IC, 448:512] = eB.T
    w[IC:C, 512:576] = eB.T
    w[0:IC, 576:704] = Wm.T
    w[IC:C, 576:704] = Wm.T
    return np.ascontiguousarray(w.astype(ml_dtypes.bfloat16))


def kernel(x, Wq, Wk, Wv, B, Wm):
    import ml_dtypes

    x = np.ascontiguousarray(np.asarray(x, dtype=np.float32))
    Wq = np.asarray(Wq, dtype=np.float32)
    Wk = np.asarray(Wk, dtype=np.float32)
    Wv = np.asarray(Wv, dtype=np.float32)
    B = np.asarray(B, dtype=np.float32)
    Wm = np.asarray(Wm, dtype=np.float32)

    xf = x.reshape(BS, C, T)
    per_batch = NCORES // BS
    shards = []
    for core in range(NCORES):
        b, j = divmod(core, per_batch)
        shards.append(np.ascontiguousarray(
            xf[b, :, j * NCOL:(j + 1) * NCOL].astype(ml_dtypes.bfloat16)))

    w = _pack_weights(Wq, Wk, Wv, B, Wm)

    if "nc" not in _cached:
        _cached["nc"] = _build()
    nc = _cached["nc"]

    in_maps = [{"x": shards[i], "w": w} for i in range(NCORES)]
    trace = bool(int(os.environ.get("AFT_TRACE", "0")))
    if trace:
        _install_ntff_hook()
    try:
        res = run_bass_kernel_spmd(
            nc, in_maps, core_ids=list(range(NCORES)), trace=trace
        )
    except Exception as e:  # rare transient device wedge: retry once
        print(f"run_bass_kernel_spmd failed ({e}); retrying", file=sys.stderr)
        import time

        time.sleep(3.0)
        res = run_bass_kernel_spmd(
            nc, in_maps, core_ids=list(range(NCORES)), trace=trace
        )
    kernel.last_exec_time_ns = res.exec_time_ns
    kernel.last_results = res

    out = np.empty((BS, C, T), np.float32)
    for core in range(NCORES):
        b, j = divmod(core, per_batch)
        out[b, :, j * NCOL:(j + 1) * NCOL] = np.asarray(
            res.results[core]["out"], dtype=np.float32)
    return out.reshape(BS, C, HH, WW)


kernel.last_exec_time_ns = None
kernel.last_results = None
